# revision 16
# baseline (speedup 1.0000x reference)
"""BiLevelGAT (2-branch x 3-layer GATv2, N=50000, E=500000, D=96) on 8 TRN2 cores.

Sharding: nodes + incoming edges partitioned by dst; per-layer AllGather of a
bf16 per-node table [hl_loc 96|1|w_loc|w_glob|pad|hl_glob 96|1|w_glob|pad]
(512B rows) gathered per edge by src (edges grouped into per-(window, stream)
sections of 128-slot blocks; stream = src side of the int16 table split).

Math: lrelu(x) = 0.6x+0.4|x| splits the GATv2 logit into linear terms (per-src
w=exp(0.6*att.hl) folded into the softmax weight; per-dst term cancels in
softmax; per-edge ea term added on device as eaT^T @ (0.6*We@att)) plus
0.4*att.|m| computed on device. Softmax max-subtraction skipped (logits O(1),
fp32 safe).

Warm-call latency is transfer-bound over the axon tunnel, so per-call upload
is kept minimal (~3MB/core): the one-hot dst scatter matrix Rt [96, NSLOT] is
built on device from a compact dcol vector (-1 marks pad slots => all-zero
one-hot column, which silently drops the slot from numerator and denominator
of the softmax); edge attrs ship dim-major bf16; x and the weight blob ship
bf16 (weights sharded 12 rows/core + device AllGather); gather indices ship
unreplicated [16, NSLOT/16] and are fanned out to 128 partitions on device.
The pjrt/shard_map executor is built once and cached across calls.
"""
import sys
sys.path.insert(0, '/opt/trn_rl_repo')
import numpy as np
import ml_dtypes

BF16 = ml_dtypes.bfloat16
FP8 = ml_dtypes.float8_e3m4

N, E, D, EDIM, L, DENSE, OUT = 50000, 500000, 96, 8, 3, 256, 2
NCORES = 8
NLOC = N // NCORES            # 6250
WIN, HALF = 96, 48
NWIN = (NLOC + WIN - 1) // WIN  # 66
NPAD = NWIN * WIN             # 6336
NCH = (NPAD + 127) // 128     # 50 chunks of 128 (PASS A / table)
SPLIT = 32768
TROW = 256

_CACHE = {}


def _host_prep(x, edge_index, edge_attr):
    src = edge_index[0].astype(np.int64)
    dst = edge_index[1].astype(np.int64)
    mean_ea = edge_attr.mean(0).astype(np.float32)
    loop = np.arange(N, dtype=np.int64)
    src_a = np.concatenate([src, loop])
    dst_a = np.concatenate([dst, loop])
    ea_a = np.concatenate([edge_attr.astype(np.float32),
                           np.broadcast_to(mean_ea, (N, EDIM))], 0)

    owner = dst_a // NLOC
    dloc = dst_a - owner * NLOC
    win = dloc // WIN
    stream = (src_a >= SPLIT).astype(np.int64)

    per_core = []
    secs = np.zeros((NCORES, NWIN, 2), np.int64)
    for c in range(NCORES):
        m = owner == c
        s_c, d_c, e_c = src_a[m], dloc[m], ea_a[m]
        w_c, st_c = win[m], stream[m]
        sec = w_c * 2 + st_c
        order = np.argsort(sec * NLOC + d_c, kind='stable')
        s_c, d_c, e_c, sec = s_c[order], d_c[order], e_c[order], sec[order]
        st_c = st_c[order]
        per_core.append((s_c, d_c, e_c, sec, st_c))
        secs[c] = np.bincount(sec, minlength=NWIN * 2).reshape(NWIN, 2)

    K = np.maximum((secs.max(0) + 127) // 128, 1)       # [NWIN, 2]
    Kf = K.reshape(-1)
    sec_slot = np.zeros(NWIN * 2 + 1, np.int64)
    np.cumsum(Kf * 128, out=sec_slot[1:])
    NSLOT = int(sec_slot[-1])
    NB = NSLOT // 128

    gidx = np.zeros((NCORES, NSLOT), np.int16)
    dc96 = np.full((NCORES, NSLOT), -1.0, np.float32)
    eaT = np.zeros((NCORES, EDIM, NSLOT), np.float32)

    for c in range(NCORES):
        s_c, d_c, e_c, sec, st_c = per_core[c]
        counts = np.bincount(sec, minlength=NWIN * 2)
        starts = np.concatenate([[0], np.cumsum(counts)])[:-1]
        pos = np.arange(len(s_c)) - starts[sec]
        slot = sec_slot[sec] + pos
        gidx[c, slot] = (s_c - st_c * SPLIT).astype(np.int16)
        dc96[c, slot] = (d_c % WIN).astype(np.float32)
        eaT[c, :, slot] = e_c

    gwc = np.zeros((NCORES, 16, NSLOT // 16), np.int16)
    for c in range(NCORES):
        gwc[c] = gidx[c].reshape(-1, 16).T
    dc_t = dc96.reshape(NCORES, NB, 128).transpose(0, 2, 1).copy()  # [128, NB]
    iotaf = np.broadcast_to(np.arange(WIN, dtype=np.float32), (128, WIN)).copy()

    return dict(K=K, Kf=Kf, sec_slot=sec_slot, NSLOT=NSLOT, NSEC=NWIN * 2,
                gw=gwc, dcol=dc_t, eaT=eaT.astype(FP8), iotaf=iotaf)


# packed-input layouts: (name, n_cols); all pieces of a blob share partition
# count and dtype.  'dcol' (w128) and 'eaT' (bf8) are per-core, rest replicated.
PACK96 = ([(f'{nm}_{l}_{b}', wd) for l in range(L) for b in range(2)
           for nm, wd in [('Wl', 96), ('Wr', 96), ('att', 1), ('bb', 1)]]
          + [('fusion_Wt', 96), ('fusion_Wb', 96), ('fusion_b', 1),
             ('pred_W1a', 128), ('pred_W1b', 128)])
W96TOT = sum(wd for _, wd in PACK96)
PACK128C = [('pred_b1a', 1), ('pred_b1b', 1),
            ('pred_W2a', 2), ('pred_W2b', 2), ('pred_b2', 2)]
PACKBF8 = ([(f'We_{l}_{b}', 96) for l in range(L) for b in range(2)] + [('v06', 6)])
BF8TOT = sum(wd for _, wd in PACKBF8)


def _wpack(w):
    o = {}
    v06 = np.zeros((EDIM, 2 * L), np.float32)
    for l in range(L):
        for b, p in enumerate(['local', 'global']):
            o[f'Wl_{l}_{b}'] = np.asarray(w[f'{p}_Wl'][l], np.float32)
            o[f'Wr_{l}_{b}'] = np.asarray(w[f'{p}_Wr'][l], np.float32)
            o[f'att_{l}_{b}'] = np.asarray(w[f'{p}_att'][l], np.float32).reshape(96, 1)
            o[f'bb_{l}_{b}'] = np.asarray(w[f'{p}_b'][l], np.float32).reshape(96, 1)
            o[f'We_{l}_{b}'] = np.asarray(w[f'{p}_We'][l], np.float32)
            v06[:, 2 * l + b] = 0.6 * (np.asarray(w[f'{p}_We'][l], np.float32)
                                       @ np.asarray(w[f'{p}_att'][l], np.float32))
    o['v06'] = v06
    o['fusion_Wt'] = np.asarray(w['fusion_W'], np.float32)[:96]
    o['fusion_Wb'] = np.asarray(w['fusion_W'], np.float32)[96:]
    o['fusion_b'] = np.asarray(w['fusion_b'], np.float32).reshape(96, 1)
    o['pred_W1a'] = np.asarray(w['pred_W1'], np.float32)[:, :128]
    o['pred_W1b'] = np.asarray(w['pred_W1'], np.float32)[:, 128:]
    b1 = np.asarray(w['pred_b1'], np.float32)
    o['pred_b1a'] = b1[:128].reshape(128, 1)
    o['pred_b1b'] = b1[128:].reshape(128, 1)
    W2 = np.asarray(w['pred_W2'], np.float32)
    o['pred_W2a'] = W2[:128]
    o['pred_W2b'] = W2[128:]
    o['pred_b2'] = np.broadcast_to(np.asarray(w['pred_b2']).reshape(1, 2), (128, 2)).astype(np.float32)
    w96 = np.concatenate([o[k] for k, _ in PACK96], axis=1)
    w128c = np.concatenate([o[k] for k, _ in PACK128C], axis=1)
    bf8 = np.concatenate([o[k] for k, _ in PACKBF8], axis=1).astype(BF16)
    return dict(w96=np.ascontiguousarray(w96.astype(BF16)), w128c=np.ascontiguousarray(w128c),
                bf8=np.ascontiguousarray(bf8))


def build_kernel(pp):
    from concourse import mybir, bacc
    import concourse.tile as tile
    Kf, sec_slot, NSLOT = pp['Kf'], pp['sec_slot'], pp['NSLOT']
    NB = NSLOT // 128
    f32, bf16, i16 = mybir.dt.float32, mybir.dt.bfloat16, mybir.dt.int16
    AF = mybir.ActivationFunctionType
    OP = mybir.AluOpType

    nc = bacc.Bacc("TRN2", target_bir_lowering=False, debug=False, num_devices=NCORES)
    dx = nc.dram_tensor("x", [NLOC, D], bf16, kind="ExternalInput")
    dgw = nc.dram_tensor("gw", [16, NSLOT // 16], i16, kind="ExternalInput")
    dw96 = nc.dram_tensor("w96", [96 // NCORES, W96TOT], bf16, kind="ExternalInput")
    dw96cp = nc.dram_tensor("w96_cp", [96 // NCORES, W96TOT], bf16)
    dw96sh = nc.dram_tensor("w96_sh", [96, W96TOT], bf16, addr_space="Shared")
    dw128 = nc.dram_tensor("w128", [128, sum(wd for _, wd in PACK128C)], f32,
                           kind="ExternalInput")
    ddcb = nc.dram_tensor("dcb", [128, NB], mybir.dt.uint8, kind="ExternalInput")
    dea8 = nc.dram_tensor("ea8", [EDIM, NSLOT], mybir.dt.float8e3, kind="ExternalInput")
    dbf8 = nc.dram_tensor("bf8", [EDIM, BF8TOT], bf16, kind="ExternalInput")
    dout = nc.dram_tensor("out", [NLOC, OUT], f32, kind="ExternalOutput")

    dRt = nc.dram_tensor("Rt_scratch", [96, NSLOT], bf16)
    tab_slice = nc.dram_tensor("tab_slice", [NLOC, TROW], bf16)
    tab_sh = nc.dram_tensor("tab_sh", [N, TROW], bf16, addr_space="Shared")
    tab = nc.dram_tensor("tab", [N, TROW], bf16)

    with tile.TileContext(nc) as tc:
      with (tc.tile_pool(name="const", bufs=1) as cp,
            tc.tile_pool(name="hp", bufs=1) as hp,
            tc.tile_pool(name="wp", bufs=1) as wp,
            tc.tile_pool(name="sp", bufs=3) as sp,
            tc.tile_pool(name="gpool", bufs=2) as gpl,
            tc.tile_pool(name="ps", bufs=2, space="PSUM") as psp,
            tc.tile_pool(name="psA", bufs=2, space="PSUM") as psA,
            tc.tile_pool(name="psagg", bufs=1, space="PSUM") as psG):

        ident = cp.tile([128, 128], bf16)
        nc.sync.dma_start(out=ident[:], in_=nc.inline_tensor(np.eye(128, dtype=BF16), name="idb").ap())
        identf = cp.tile([128, 128], f32)
        nc.sync.dma_start(out=identf[:], in_=nc.inline_tensor(np.eye(128, dtype=np.float32), name="idf").ap())
        gw_t = cp.tile([128, NSLOT // 16], i16)
        for k8 in range(8):
            nc.sync.dma_start(out=gw_t[16 * k8:16 * (k8 + 1), :], in_=dgw[:])
        wt = {}
        nc.sync.dma_start(out=dw96cp[:], in_=dw96[:])
        nc.gpsimd.collective_compute(
            "AllGather", mybir.AluOpType.bypass,
            replica_groups=[list(range(NCORES))],
            ins=[dw96cp[:]], outs=[dw96sh[:]],
        )
        w96b = cp.tile([96, W96TOT], bf16, tag="w96b", name="w96b")
        nc.sync.dma_start(out=w96b[:], in_=dw96sh[:])
        w96f = cp.tile([96, W96TOT], f32, tag="w96f", name="w96f")
        nc.vector.tensor_copy(out=w96f[:], in_=w96b[:])
        off = 0
        for k, wd in PACK96:
            wt[k] = w96f[:, off:off + wd]
            off += wd
        dcb = cp.tile([128, NB], mybir.dt.uint8)
        nc.sync.dma_start(out=dcb[:], in_=ddcb[:])
        dc_t = cp.tile([128, NB], f32)
        nc.vector.tensor_copy(out=dc_t[:], in_=dcb[:])
        off = 0
        for k, wd in PACK128C:
            wt[k] = cp.tile([128, wd], f32, tag=k, name=k)
            nc.sync.dma_start(out=wt[k][:], in_=dw128[:, off:off + wd])
            off += wd
        iota_i = cp.tile([128, WIN], mybir.dt.int32)
        nc.gpsimd.iota(iota_i[:], pattern=[[1, WIN]], base=0, channel_multiplier=0)
        iota_t = cp.tile([128, WIN], f32)
        nc.vector.tensor_copy(out=iota_t[:], in_=iota_i[:])
        off = 0
        for k, wd in PACKBF8:
            wt[k] = cp.tile([EDIM, wd], bf16, tag=k, name=k)
            nc.sync.dma_start(out=wt[k][:], in_=dbf8[:, off:off + wd])
            off += wd
        one1 = cp.tile([1, 96], f32)
        nc.vector.memset(one1[:], 1.0)
        att04 = {}
        for l in range(L):
            for b in range(2):
                att04[(l, b)] = cp.tile([96, 1], bf16, tag=f"att04_{l}_{b}", name=f"att04_{l}_{b}")
                nc.vector.tensor_scalar(out=att04[(l, b)][:], in0=wt[f'att_{l}_{b}'][:],
                                        scalar1=0.4, scalar2=None, op0=OP.mult)

        # ---------- build Rt (one-hot dst scatter, layer-invariant) ----------
        for blk in range(NB):
            oh = sp.tile([128, WIN], f32, tag="oh")
            nc.vector.tensor_scalar(out=oh[:], in0=iota_t[:], scalar1=dc_t[:, blk:blk + 1],
                                    scalar2=None, op0=OP.is_equal)
            pt = psA.tile([96, 128], f32, tag="pbig")
            nc.tensor.transpose(out=pt[:], in_=oh[:], identity=identf[:])
            rs = sp.tile([96, 128], bf16, tag="rts")
            nc.vector.tensor_copy(out=rs[:], in_=pt[:])
            nc.sync.dma_start(out=dRt[:, blk * 128:(blk + 1) * 128], in_=rs[:])

        # h_T feature-major [96, NPAD] (cols beyond NLOC are pad)
        h_T = [hp.tile([96, NCH * 128], f32, tag=f"h{b}", name=f"h{b}") for b in range(2)]
        for ch in range(NCH):
            n0 = ch * 128
            nreal = max(0, min(NLOC - n0, 128))
            xin = sp.tile([128, 128], f32, tag="xin")
            nc.vector.memset(xin[:], 0.0)
            if nreal > 0:
                xb = sp.tile([128, 96], bf16, tag="xb")
                nc.sync.dma_start(out=xb[:nreal, :], in_=dx[n0:n0 + nreal, :])
                nc.vector.tensor_copy(out=xin[:nreal, :96], in_=xb[:nreal, :])
            pt = psA.tile([128, 128], f32, tag="pbig")
            nc.tensor.transpose(out=pt[:], in_=xin[:], identity=identf[:])
            for b in range(2):
                nc.vector.tensor_copy(out=h_T[b][:, n0:n0 + 128], in_=pt[:96, :])

        hw_T = [wp.tile([96, NCH * 128], f32, tag=f"hw{b}", name=f"hw{b}") for b in range(2)]

        for l in range(L):
            # ---------- PASS A ----------
            for b in range(2):
                for cs in range(0, NCH * 128, 512):
                    ce = min(cs + 512, NCH * 128)
                    w_ = ce - cs
                    pl = psA.tile([96, 512], f32, tag="pbig")
                    nc.tensor.matmul(out=pl[:, :w_], lhsT=wt[f'Wl_{l}_{b}'][:],
                                     rhs=h_T[b][:, cs:ce], start=True, stop=True)
                    nc.vector.tensor_copy(out=hw_T[b][:, cs:ce], in_=pl[:, :w_])
            # table slice + allgather
            for ch in range(NCH):
                n0 = ch * 128
                nreal = max(0, min(NLOC - n0, 128))
                if nreal == 0:
                    continue
                stg = sp.tile([128, TROW], bf16, tag="stg")
                nc.vector.memset(stg[:], 0.0)
                for b in range(2):
                    pt = psA.tile([128, 128], f32, tag="pbig")
                    nc.tensor.transpose(out=pt[:, :96], in_=hw_T[b][:, n0:n0 + 128],
                                        identity=identf[:96, :96])
                    nc.vector.tensor_copy(out=stg[:, b * 128:b * 128 + 96], in_=pt[:, :96])
                    # w = exp(0.6*att.hl) for this chunk; ones at ext row 32
                    pphi = psA.tile([1, 128], f32, tag="pbig")
                    nc.tensor.matmul(out=pphi[:], lhsT=wt[f'att_{l}_{b}'][:],
                                     rhs=hw_T[b][:, n0:n0 + 128], start=True, stop=True)
                    ext = sp.tile([64, 128], f32, tag="ext")
                    nc.scalar.activation(out=ext[0:1, :], in_=pphi[:], func=AF.Exp, scale=0.6)
                    nc.vector.memset(ext[32:33, :], 1.0)
                    pt2 = psA.tile([128, 64], f32, tag="pbig")
                    nc.tensor.transpose(out=pt2[:], in_=ext[:], identity=identf[:64, :64])
                    nc.vector.tensor_copy(out=stg[:, b * 128 + 96:b * 128 + 97], in_=pt2[:, 32:33])
                    nc.vector.tensor_copy(out=stg[:, b * 128 + 97:b * 128 + 98], in_=pt2[:, 0:1])
                nc.vector.tensor_copy(out=stg[:, 98:99], in_=stg[:, 225:226])
                nc.sync.dma_start(out=tab_slice[n0:n0 + nreal, :], in_=stg[:nreal, :])
            nc.gpsimd.collective_compute(
                "AllGather", mybir.AluOpType.bypass,
                replica_groups=[list(range(NCORES))],
                ins=[tab_slice[:]], outs=[tab_sh[:]],
            )
            nc.sync.dma_start(out=tab[:], in_=tab_sh[:])

            # ---------- edge phase ----------
            for w in range(NWIN):
                aggp = {}
                first = {b: True for b in range(2)}
                nagg = {b: 0 for b in range(2)}
                tot = {b: sum(int(Kf[w * 2 + s]) for s in range(2)) for b in range(2)}
                for b in range(2):
                    aggp[b] = psG.tile([97, WIN], f32, tag=f"agg{b}", name=f"agg{b}")
                # base lhsT per branch for this window (hr = h @ Wr computed here)
                basel = {}
                for b in range(2):
                    phr = psA.tile([96, WIN], f32, tag="pbig")
                    nc.tensor.matmul(out=phr[:], lhsT=wt[f'Wr_{l}_{b}'][:],
                                     rhs=h_T[b][:, w * WIN:(w + 1) * WIN],
                                     start=True, stop=True)
                    hrs = sp.tile([96, WIN], f32, tag="hrs")
                    nc.vector.tensor_copy(out=hrs[:], in_=phr[:])
                    pt = psA.tile([WIN, 96], f32, tag="pbig")
                    nc.tensor.transpose(out=pt[:], in_=hrs[:], identity=identf[:96, :96])
                    bl = sp.tile([WIN, 96], bf16, tag=f"basel{b}", name=f"basel{b}")
                    nc.vector.tensor_copy(out=bl[:], in_=pt[:])
                    basel[b] = bl
                for s in range(2):
                        si = w * 2 + s
                        Ks = int(Kf[si])
                        sl0 = int(sec_slot[si])
                        nsl = Ks * 128
                        g = gpl.tile([128, 7, TROW], bf16, tag="gath")
                        nc.gpsimd.dma_gather(
                            out_ap=g[:, :Ks, :],
                            in_ap=tab[SPLIT:, :] if s else tab[:SPLIT, :],
                            idxs_ap=gw_t[:, sl0 // 16:(sl0 + nsl) // 16],
                            num_idxs=nsl, num_idxs_reg=nsl, elem_size=TROW)
                        Rt = sp.tile([96, 7 * 128], bf16, tag="Rt")
                        nc.sync.dma_start(out=Rt[:, :nsl], in_=dRt[:, sl0:sl0 + nsl])
                        eas = sp.tile([EDIM, 7 * 128], mybir.dt.float8e3, tag="eas")
                        nc.sync.dma_start(out=eas[:, :nsl], in_=dea8[:, sl0:sl0 + nsl])
                        lgp = psp.tile([128, 16], f32, tag="lgp", bufs=1)
                        for j0 in range(0, Ks, 4):
                            jw = min(4, Ks - j0)
                            for b in range(2):
                                mps = psp.tile([96, 512], f32, tag="mps")
                                nc.tensor.matmul(out=mps[:, :jw * 128], lhsT=basel[b][:],
                                                 rhs=Rt[:, j0 * 128:(j0 + jw) * 128],
                                                 start=True, stop=False)
                                nc.tensor.matmul(out=mps[:, :jw * 128], lhsT=wt[f'We_{l}_{b}'][:],
                                                 rhs=eas[:, j0 * 128:(j0 + jw) * 128],
                                                 start=False, stop=False,
                                                 skip_group_check=True)
                                for dj in range(jw):
                                    j = j0 + dj
                                    nc.tensor.matmul(out=mps[:, dj * 128:(dj + 1) * 128],
                                                     lhsT=g[:, j, b * 128:b * 128 + 96],
                                                     rhs=ident[:], start=False,
                                                     stop=(dj == jw - 1),
                                                     skip_group_check=True)
                                am = sp.tile([96, 512], bf16, tag="am")
                                nc.scalar.activation(out=am[:, :jw * 128],
                                                     in_=mps[:, :jw * 128], func=AF.Abs)
                                for dj in range(jw):
                                    j = j0 + dj
                                    nc.tensor.matmul(out=lgp[:, 2 * j + b:2 * j + b + 1],
                                                     lhsT=am[:, dj * 128:(dj + 1) * 128],
                                                     rhs=att04[(l, b)][:],
                                                     start=(j == 0 and b == 0), stop=False,
                                                     skip_group_check=True)
                        for j in range(Ks):
                            nc.tensor.matmul(out=lgp[:, 2 * j:2 * j + 2],
                                             lhsT=eas[:, j * 128:(j + 1) * 128],
                                             rhs=wt['v06'][:, 2 * l:2 * l + 2],
                                             start=False, stop=(j == Ks - 1),
                                             skip_group_check=True)
                        exw = sp.tile([128, 16], f32, tag="exw")
                        nc.scalar.activation(out=exw[:, :2 * Ks], in_=lgp[:, :2 * Ks],
                                             func=AF.Exp)
                        nc.vector.tensor_tensor(
                            out=exw[:, :2 * Ks].rearrange("p (j b) -> p j b", b=2),
                            in0=exw[:, :2 * Ks].rearrange("p (j b) -> p j b", b=2),
                            in1=g[:, :Ks, 97:99], op=OP.mult)
                        for j in range(Ks):
                            blk = sl0 // 128 + j
                            for b in range(2):
                                es = sp.tile([128, WIN], bf16, tag="es")
                                nc.vector.tensor_scalar(
                                    out=es[:], in0=iota_t[:],
                                    scalar1=dc_t[:, blk:blk + 1],
                                    scalar2=exw[:, 2 * j + b:2 * j + b + 1],
                                    op0=OP.is_equal, op1=OP.mult)
                                nagg[b] += 1
                                nc.tensor.matmul(out=aggp[b][:],
                                                 lhsT=g[:, j, b * 128:b * 128 + 97],
                                                 rhs=es[:],
                                                 start=first[b], stop=(nagg[b] == tot[b]),
                                                 skip_group_check=True)
                                first[b] = False
                # finalize window -> h_T
                for b in range(2):
                    num = sp.tile([96, WIN], f32, tag="num")
                    den = sp.tile([1, WIN], f32, tag="den")
                    nc.vector.tensor_copy(out=num[:], in_=aggp[b][:96, :])
                    nc.vector.tensor_scalar(out=den[:], in0=aggp[b][96:97, :],
                                            scalar1=1e-30, scalar2=None, op0=OP.add)
                    rec = sp.tile([1, WIN], f32, tag="rec")
                    nc.vector.reciprocal(out=rec[:], in_=den[:])
                    pb = psp.tile([96, WIN], f32, tag="mps")
                    nc.tensor.matmul(out=pb[:], lhsT=one1[:], rhs=rec[:], start=True, stop=True)
                    tdiv = sp.tile([96, WIN], f32, tag="tdiv")
                    nc.vector.tensor_tensor(out=tdiv[:], in0=num[:], in1=pb[:], op=OP.mult)
                    lin = sp.tile([96, WIN], f32, tag="lin")
                    nc.scalar.activation(out=lin[:], in_=tdiv[:], func=AF.Identity,
                                         bias=wt[f'bb_{l}_{b}'][:])
                    ab = sp.tile([96, WIN], f32, tag="ab")
                    nc.scalar.activation(out=ab[:], in_=tdiv[:], func=AF.Abs,
                                         bias=wt[f'bb_{l}_{b}'][:])
                    nc.vector.tensor_scalar(out=lin[:], in0=lin[:], scalar1=0.505,
                                            scalar2=None, op0=OP.mult)
                    nc.vector.tensor_scalar(out=ab[:], in0=ab[:], scalar1=0.495,
                                            scalar2=None, op0=OP.mult)
                    nc.vector.tensor_tensor(out=h_T[b][:, w * WIN:(w + 1) * WIN],
                                            in0=lin[:], in1=ab[:], op=OP.add)

        # ---------- head ----------
        hid_T = [wp.tile([128, NCH * 128], f32, tag=f"hw{p}", name=f"hid{p}") for p in range(2)]
        for cs in range(0, NCH * 128, 512):
            ce = min(cs + 512, NCH * 128)
            w_ = ce - cs
            pf = psA.tile([96, 512], f32, tag="pbig")
            nc.tensor.matmul(out=pf[:, :w_], lhsT=wt['fusion_Wt'][:],
                             rhs=h_T[0][:, cs:ce], start=True, stop=False)
            nc.tensor.matmul(out=pf[:, :w_], lhsT=wt['fusion_Wb'][:],
                             rhs=h_T[1][:, cs:ce], start=False, stop=True)
            fus = sp.tile([96, 512], f32, tag="fus")
            lin = sp.tile([96, 512], f32, tag="flin")
            nc.scalar.activation(out=lin[:, :w_], in_=pf[:, :w_], func=AF.Identity,
                                 bias=wt['fusion_b'][:])
            ab = sp.tile([96, 512], f32, tag="fab")
            nc.scalar.activation(out=ab[:, :w_], in_=pf[:, :w_], func=AF.Abs,
                                 bias=wt['fusion_b'][:])
            nc.vector.tensor_scalar(out=lin[:, :w_], in0=lin[:, :w_], scalar1=0.505,
                                    scalar2=None, op0=OP.mult)
            nc.vector.tensor_scalar(out=ab[:, :w_], in0=ab[:, :w_], scalar1=0.495,
                                    scalar2=None, op0=OP.mult)
            nc.vector.tensor_tensor(out=fus[:, :w_], in0=lin[:, :w_], in1=ab[:, :w_],
                                    op=OP.add)
            for p, (wk, bk) in enumerate([('pred_W1a', 'pred_b1a'), ('pred_W1b', 'pred_b1b')]):
                ph = psA.tile([128, 512], f32, tag="pbig")
                nc.tensor.matmul(out=ph[:, :w_], lhsT=wt[wk][:], rhs=fus[:, :w_],
                                 start=True, stop=True)
                l2 = sp.tile([128, 512], f32, tag=f"l2{p}")
                a2 = sp.tile([128, 512], f32, tag=f"a2{p}")
                nc.scalar.activation(out=l2[:, :w_], in_=ph[:, :w_], func=AF.Identity,
                                     bias=wt[bk][:])
                nc.scalar.activation(out=a2[:, :w_], in_=ph[:, :w_], func=AF.Abs,
                                     bias=wt[bk][:])
                nc.vector.tensor_scalar(out=l2[:, :w_], in0=l2[:, :w_], scalar1=0.505,
                                        scalar2=None, op0=OP.mult)
                nc.vector.tensor_scalar(out=a2[:, :w_], in0=a2[:, :w_], scalar1=0.495,
                                        scalar2=None, op0=OP.mult)
                nc.vector.tensor_tensor(out=hid_T[p][:, cs:ce], in0=l2[:, :w_],
                                        in1=a2[:, :w_], op=OP.add)
        for ch in range(NCH):
            n0 = ch * 128
            nreal = max(0, min(NLOC - n0, 128))
            if nreal == 0:
                continue
            po = psp.tile([128, 2], f32, tag="mps")
            nc.tensor.matmul(out=po[:], lhsT=hid_T[0][:, n0:n0 + 128],
                             rhs=wt['pred_W2a'][:], start=True, stop=False)
            nc.tensor.matmul(out=po[:], lhsT=hid_T[1][:, n0:n0 + 128],
                             rhs=wt['pred_W2b'][:], start=False, stop=True)
            ot = sp.tile([128, 2], f32, tag="ot")
            nc.vector.tensor_tensor(out=ot[:], in0=po[:], in1=wt['pred_b2'][:], op=OP.add)
            nc.sync.dma_start(out=dout[n0:n0 + nreal, :], in_=ot[:nreal, :])

    nc.compile()
    return nc


def _make_runner(nc):
    """Build the pjrt/shard_map executor once; reuse across calls.

    Same execution primitive run_bass_kernel_spmd uses under axon
    (bass2jax._bass_exec_p via jit(shard_map)), but with the traced/compiled
    wrapper cached so warm calls pay only input transfer + device exec.
    """
    import jax
    from jax.sharding import Mesh, PartitionSpec
    try:
        from jax.experimental.shard_map import shard_map
    except ImportError:
        from jax import shard_map
    from concourse import mybir
    from concourse.bass2jax import _bass_exec_p, partition_id_tensor, install_neuronx_cc_hook
    install_neuronx_cc_hook()

    partition_name = nc.partition_id_tensor.name if nc.partition_id_tensor else None
    in_names, out_names, out_avals, zero_shapes = [], [], [], []
    for alloc in nc.m.functions[0].allocations:
        if not isinstance(alloc, mybir.MemoryLocationSet):
            continue
        name = alloc.memorylocations[0].name
        if alloc.kind == "ExternalInput":
            if name != partition_name:
                in_names.append(name)
        elif alloc.kind == "ExternalOutput":
            shape = tuple(alloc.tensor_shape)
            dtype = mybir.dt.np(alloc.dtype)
            out_names.append(name)
            out_avals.append(jax.core.ShapedArray(shape, dtype))
            zero_shapes.append((shape, dtype))
    n_params, n_outs = len(in_names), len(out_names)
    names_all = in_names + out_names + ([partition_name] if partition_name else [])

    def _body(*args):
        operands = list(args)
        if partition_name is not None:
            operands.append(partition_id_tensor())
        return tuple(_bass_exec_p.bind(
            *operands, out_avals=tuple(out_avals), in_names=tuple(names_all),
            out_names=tuple(out_names), lowering_input_output_aliases=(),
            sim_require_finite=True, sim_require_nnan=True, nc=nc))

    devices = jax.devices()[:NCORES]
    mesh = Mesh(np.asarray(devices), ("core",))
    sharded = jax.jit(
        shard_map(_body, mesh=mesh,
                  in_specs=(PartitionSpec("core"),) * (n_params + n_outs),
                  out_specs=(PartitionSpec("core"),) * n_outs, check_rep=False),
        donate_argnums=tuple(range(n_params, n_params + n_outs)), keep_unused=True)

    def run(in_maps):
        concat_in = [np.concatenate([np.asarray(m[nm]) for m in in_maps], axis=0)
                     for nm in in_names]
        zs = [np.zeros((NCORES * s[0], *s[1:]), dt) for (s, dt) in zero_shapes]
        outs = sharded(*concat_in, *zs)
        return {nm: np.asarray(outs[i]).reshape(NCORES, *out_avals[i].shape)
                for i, nm in enumerate(out_names)}
    return run


def _make_in_maps(x, pp, wpk):
    xb = np.asarray(x, np.float32).astype(BF16)
    in_maps = []
    for c in range(NCORES):
        m = {'x': np.ascontiguousarray(xb[c * NLOC:(c + 1) * NLOC]),
             'gw': np.ascontiguousarray(pp['gw'][c]),
             'w96': np.ascontiguousarray(wpk['w96'][c * (96 // NCORES):(c + 1) * (96 // NCORES)]),
             'w128': wpk['w128c'],
             'dcb': np.ascontiguousarray(
                 np.where(pp['dcol'][c] < 0, 255, pp['dcol'][c]).astype(np.uint8)),
             'ea8': np.ascontiguousarray(pp['eaT'][c]),
             'bf8': wpk['bf8']}
        in_maps.append(m)
    return in_maps


def kernel(**inputs):
    import hashlib
    x = np.asarray(inputs['x'], np.float32)
    ei = np.asarray(inputs['edge_index'])
    ea = np.asarray(inputs['edge_attr'], np.float32)
    gkey = (hashlib.md5(np.ascontiguousarray(ei)).hexdigest(),
            hashlib.md5(np.ascontiguousarray(ea)).hexdigest())
    if _CACHE.get('gkey') != gkey:
        _CACHE['pp'] = _host_prep(x, ei, ea)
        _CACHE['gkey'] = gkey
    pp = _CACHE['pp']
    ckey = (pp['NSLOT'], pp['Kf'].tobytes())
    if _CACHE.get('ckey') != ckey:
        nc = build_kernel(pp)
        _CACHE.update(ckey=ckey, nc=nc, runner=_make_runner(nc))
    wpk = _wpack(inputs)
    res = _CACHE['runner'](_make_in_maps(x, pp, wpk))
    return res['out'].reshape(N, OUT).astype(np.float32)
